# revision 26
# baseline (speedup 1.0000x reference)
"""KAN layer (B-spline + silu) Trainium2 kernel, 8-way tensor-parallel.

Math (uniform knot grid):
  Truncated-power features S_i(v) = relu(v - i)^3, v = (x - t0)/h, i = 0..14,
  are computed in f32 on the scalar/vector engines (relu -> square -> cube).
  A small banded f32 matmul on the PE ("combine") turns them into the local
  B-spline basis  B_f = sum_{r=0..4} w5[r] * S_{f+r},  f = 0..10  (w5 =
  [1,-4,6,-4,1]/6).  The combine must run in f32: the truncated powers (up
  to ~2000) cancel down to B <= 0.67.  Its output is post-cancellation, so
  it is cast to fp16, and the main matmul runs fully in fp16 (1 PE
  cycle/row instead of 4 for f32):
      out[n, j*256+q] = sum_f B_f(v[n,j]) * Cw[f, j*256+q]
                        + silu(x[n,j]) * W[j*256+q],   Cw = C * W.
  fp16 scaling: weights are stored as 32*Cw / 32*W (lifting them out of the
  fp16 subnormal range) and the basis as B/32, silu/32 — the f32 PSUM result
  is the unscaled output.  The output is written to HBM in fp16 (halving the
  HBM-write floor, which dominates) and widened to f32 on the host.

Sharding: core s owns j in [32s, 32s+32) (columns [8192s, 8192(s+1)) of the
flattened output).  Per core, j's are grouped into 4 octets of 8; within an
octet, j-pairs map to the 4 PE row groups.  Row layout per 32-row group:
  S tile (f32):  [15 S(j_a), 15 S(j_b), silu'(j_a), silu'(j_b)]
  B tile (fp16): [11 B'(j_a), 11 B'(j_b), silu'(j_a), silu'(j_b), 8 zeros]
The combine matmul (K=32 -> M=32, tile_position (32r,32r), silu rows passed
through, last 8 out-cols zero) and the main matmul (K=32, rhs rows 24..31
zero, tile_position (32r,0)) both use full 32-row groups.

Performance structure (per core):
  - n is processed in 4 chunks of 512 rows; within a chunk, partition p
    holds output rows 512c + 4p + t (t = 0..3) — the main matmuls take
    stride-4 column slices of the B tile — so each store is 16 KB/partition
    (16 KB DMA packets, near-peak HBM write rate).
  - The four row-group matmuls of one parity issue back-to-back to distinct
    tile_positions (4-way concurrent on the PE); PSUM is one pool of four
    2-bank tiles shared by combine and main.
  - PSUM evacuation (f32 -> fp16 copy, the true output) alternates between
    scalar and vector; the chain is split (relu+square on scalar, cube on
    vector) to balance them, and chain ops run as half-tiles so they never
    displace more than ~1us of copies.
  - Input DMAs and octet 1-3 silu scatters ride the gpsimd queue, octet-0
    scatters the scalar queue, output stores own the sync queue.
  - Emission is a software-pipelined wavefront: chains 0 and 1 run entirely
    up-front (in engine time that is idle anyway while the first chunks
    compute), chain 2 is spread through octet 0, chain 3 through octet 1;
    combine piece q feeds main chunk q immediately.
"""

import numpy as np

import concourse.bass as bass
import concourse.bacc as bacc
import concourse.tile as tile
from concourse import mybir
from concourse.bass_utils import run_bass_kernel_spmd

N = 2048          # batch
N_IN = 256
N_OUT = 256
NCORES = 8
JPC = N_IN // NCORES      # 32 j per core
NOCT = JPC // 8           # 4 octets of 8 j's
NCH = N // 512            # 4 n-chunks of 512 rows
NQ = N // 512             # 4 combine pieces along n (piece q == chunk q)
F32 = mybir.dt.float32
F16 = mybir.dt.float16
WSCALE = 32.0             # fp16 weight scale (basis/silu carry 1/32)


def _build_bass(scale_val: float):
    nc = bacc.Bacc(trn_type="TRN2")

    xrep = nc.dram_tensor("xrep", [NOCT, 128, N], F16, kind="ExternalInput")
    biasv = nc.dram_tensor("biasv", [128, 1], F32, kind="ExternalInput")
    w5b = nc.dram_tensor("w5b", [128, 32], F32, kind="ExternalInput")
    rhsbd = nc.dram_tensor("rhsbd", [128, NOCT * 512], F16, kind="ExternalInput")
    siluT = nc.dram_tensor("siluT", [JPC, N], F32, kind="ExternalInput")
    # out[o, c, p, t*2048 + j] = row n = 512c + 4p + t, col 2048o + j
    out = nc.dram_tensor("out", [NOCT, NCH, 128, 8192], F16,
                         kind="ExternalOutput")

    with tile.TileContext(nc) as tc:
        with (
            tc.tile_pool(name="consts", bufs=1) as consts,
            tc.tile_pool(name="xin", bufs=4) as xin,
            tc.tile_pool(name="chain", bufs=2) as chain,
            tc.tile_pool(name="ss", bufs=1) as sspool,
            tc.tile_pool(name="bsb", bufs=1) as bpool,
            tc.tile_pool(name="stage", bufs=3) as stage_pool,
            tc.tile_pool(name="psum", bufs=4, space="PSUM") as psum_pool,
        ):
            # Input loads ride the gpsimd queue (sync = stores, scalar/vector
            # = compute + octet-0 scatters).
            bias_sb = consts.tile([128, 1], F32, name="bias_sb")
            nc.gpsimd.dma_start(out=bias_sb, in_=biasv[:, :])
            xr_tiles = []
            for o in range(NOCT):
                xr = xin.tile([128, N], F16, tag=f"xr{o}", name=f"xr{o}")
                xr_tiles.append(xr)
            nc.gpsimd.dma_start(out=xr_tiles[0], in_=xrep[0])
            nc.gpsimd.dma_start(out=xr_tiles[1], in_=xrep[1])
            rhs_sb = consts.tile([128, NOCT * 512], F16, name="rhs_sb")
            nc.gpsimd.dma_start(out=rhs_sb, in_=rhsbd[:, :])
            w5b_sb = consts.tile([128, 32], F32, name="w5b_sb")
            nc.gpsimd.dma_start(out=w5b_sb, in_=w5b[:, :])
            for o in range(2, NOCT):
                nc.gpsimd.dma_start(out=xr_tiles[o], in_=xrep[o])

            ss_tiles = [None] * NOCT
            b_tiles = [None] * NOCT
            chain_t = [None] * NOCT
            cnt = 0

            def emit_chain_step(o, step):
                # 6 half-tile steps ([128, 1024] each):
                #   0/1: relu halves (scalar), 2/3: square halves (scalar),
                #   4/5: cube halves (vector); silu scatters after step 5
                #   (gpsimd queue except octet 0).
                h = N // 2
                lo, hi = (0, h) if step % 2 == 0 else (h, N)
                if step < 2:
                    if step == 0:
                        t1 = chain.tile([128, N], F32, tag="t1", name=f"t1_{o}")
                        chain_t[o] = t1
                    t1 = chain_t[o]
                    nc.scalar.activation(
                        t1[:, lo:hi], xr_tiles[o][:, lo:hi],
                        mybir.ActivationFunctionType.Relu,
                        bias=bias_sb[:, 0:1], scale=scale_val,
                    )
                elif step < 4:
                    if step == 2:
                        t1 = chain_t[o]
                        t2 = chain.tile([128, N], F32, tag="t2", name=f"t2_{o}")
                        chain_t[o] = (t1, t2)
                    t1, t2 = chain_t[o]
                    nc.scalar.square(t2[:, lo:hi], t1[:, lo:hi])
                else:
                    if step == 4:
                        ss_tiles[o] = sspool.tile([128, N], F32, tag=f"ss{o}",
                                                  name=f"ss{o}")
                    t1, t2 = chain_t[o]
                    ss = ss_tiles[o]
                    nc.vector.tensor_mul(ss[:, lo:hi], t1[:, lo:hi],
                                         t2[:, lo:hi])
                    if step == 5:
                        eng = nc.scalar if o == 0 else nc.gpsimd
                        for r in range(4):
                            eng.dma_start(
                                out=ss[32 * r + 30 : 32 * r + 32, :],
                                in_=siluT[8 * o + 2 * r : 8 * o + 2 * r + 2, :],
                            )

            def emit_combine_piece(o, q):
                if q == 0:
                    b_tiles[o] = bpool.tile([128, N], F16, tag=f"b{o}",
                                            name=f"b{o}")
                bsb = b_tiles[o]
                bpt = psum_pool.tile([128, 1024], F32, tag="ps",
                                     name=f"bps{o}_{q}")
                bps = bpt[:, 0:512]
                for r in range(4):
                    nc.tensor.matmul(
                        bps[32 * r : 32 * r + 32, :],
                        lhsT=w5b_sb[32 * r : 32 * r + 32, :],
                        rhs=ss_tiles[o][32 * r : 32 * r + 32,
                                        512 * q : 512 * (q + 1)],
                        start=True,
                        stop=True,
                        tile_position=(32 * r, 32 * r),
                    )
                dst = bsb[:, 512 * q : 512 * (q + 1)]
                if q % 2 == 0:
                    nc.vector.tensor_scalar_mul(dst, bps, 1.0)
                else:
                    nc.scalar.copy(dst, bps)

            def emit_main_chunk(o, c):
                nonlocal cnt
                bsb = b_tiles[o]
                st = stage_pool.tile([128, 8192], F16, tag="st",
                                     name=f"st{o}_{c}")
                for t in range(4):       # row residue: n = 512c + 4p + t
                    for rp in range(2):  # row-group pairs (2rp, 2rp+1)
                        ps = psum_pool.tile([128, 1024], F32, tag="ps",
                                            name=f"ps{o}_{c}_{t}_{rp}")
                        for rr in range(2):
                            r = 2 * rp + rr
                            nc.tensor.matmul(
                                ps[:, 512 * rr : 512 * (rr + 1)],
                                lhsT=bsb[32 * r : 32 * r + 32,
                                         512 * c + t : 512 * (c + 1) : 4],
                                rhs=rhs_sb[32 * r : 32 * r + 32,
                                           512 * o : 512 * (o + 1)],
                                start=True,
                                stop=True,
                                tile_position=(32 * r, 0),
                            )
                        dst = st[:, 2048 * t + 1024 * rp :
                                 2048 * t + 1024 * (rp + 1)]
                        if cnt % 2 == 0:
                            nc.vector.tensor_scalar_mul(dst, ps, 1.0)
                        else:
                            nc.scalar.copy(dst, ps)
                        cnt += 1
                nc.sync.dma_start(out=out[o, c], in_=st)

            # Software-pipelined wavefront.
            for s in range(6):
                emit_chain_step(0, s)
            for s in range(6):
                emit_chain_step(1, s)
            for o in range(NOCT):
                for c in range(NCH):
                    co = o + 2  # chain for octet o+2 spreads over octet o
                    if co < NOCT:
                        for s in (2 * c - 1, 2 * c):
                            if 0 <= s < 6:
                                emit_chain_step(co, s)
                    emit_combine_piece(o, c)
                    emit_main_chunk(o, c)

    nc.compile()
    return nc


def _host_prep(x, C, W, grid):
    """Build per-core input maps."""
    t0 = np.float64(grid[0, 0])
    h = np.float64(grid[0, 1] - grid[0, 0])
    w5 = np.array([1.0, -4.0, 6.0, -4.0, 1.0], np.float64) / 6.0

    # Banded combine weights (f32): B'_f = sum_r (w5[r]/32) S_{f+r} for both
    # j's of the pair, silu pass-through rows 30/31 -> 22/23, cols 24..31 = 0.
    w5b1 = np.zeros((32, 32), np.float32)
    for f in range(11):
        for r in range(5):
            w5b1[f + r, f] = np.float32(w5[r] / WSCALE)
            w5b1[15 + f + r, 11 + f] = np.float32(w5[r] / WSCALE)
    w5b1[30, 22] = 1.0
    w5b1[31, 23] = 1.0
    w5b = np.ascontiguousarray(np.tile(w5b1, (4, 1)))  # same block per group

    Cw32 = (C.astype(np.float64) * W.astype(np.float64) * WSCALE).astype(np.float16)
    W32 = (W.astype(np.float64) * WSCALE).astype(np.float16)

    xd = x.astype(np.float64)
    silu_p = (xd / (1.0 + np.exp(-xd)) / WSCALE).astype(np.float32)  # silu/32

    # S-tile partition layout within a 32-row group:
    #   s in [0,15)  -> S_i of j_a (i = s)
    #   s in [15,30) -> S_i of j_b (i = s - 15)
    #   s = 30/31    -> silu'(j_a)/silu'(j_b) (scatter; relu bias -64 ->
    #                   the chain writes exact zeros there first)
    s_idx = np.arange(128) % 32
    feat_i = np.where(s_idx < 15, s_idx, np.where(s_idx < 30, s_idx - 15, 0))
    which_b = np.where(s_idx < 15, 0, np.where(s_idx < 30, 1, s_idx - 30))
    biasv = np.where(
        s_idx < 30, -t0 / h - feat_i, -64.0
    ).astype(np.float32).reshape(128, 1)
    scale_val = float(np.float32(1.0 / h))

    x16 = x.astype(np.float16)
    in_maps = []
    for s in range(NCORES):
        jb = JPC * s
        xt = np.ascontiguousarray(x16[:, jb : jb + JPC].T)    # (32, N) fp16
        xrep = np.empty((NOCT, 128, N), np.float16)
        rgrp = np.arange(128) // 32
        for o in range(NOCT):
            jloc = 8 * o + 2 * rgrp + which_b
            xrep[o] = xt[jloc]
        silu_t = np.ascontiguousarray(silu_p[:, jb : jb + JPC].T)  # (32, N) f32

        # B-tile row layout per group: [11 B'a, 11 B'b, silu'a, silu'b, 8 pad]
        rhsbd = np.zeros((128, NOCT * 512), np.float16)
        for o in range(NOCT):
            for rr in range(4):
                ja = (jb + 8 * o + 2 * rr) * N_OUT
                jbc = (jb + 8 * o + 2 * rr + 1) * N_OUT
                base = 32 * rr
                rhsbd[base : base + 11, 512 * o : 512 * o + 256] = \
                    Cw32[:, ja : ja + 256]
                rhsbd[base + 11 : base + 22, 512 * o + 256 : 512 * o + 512] = \
                    Cw32[:, jbc : jbc + 256]
                rhsbd[base + 22, 512 * o : 512 * o + 256] = W32[0, ja : ja + 256]
                rhsbd[base + 23, 512 * o + 256 : 512 * o + 512] = \
                    W32[0, jbc : jbc + 256]
        in_maps.append({
            "xrep": np.ascontiguousarray(xrep),
            "biasv": biasv,
            "w5b": w5b,
            "rhsbd": np.ascontiguousarray(rhsbd),
            "siluT": silu_t,
        })
    return in_maps, scale_val


def _assemble(out_core):
    """[NOCT, NCH, 128, 8192] fp16 -> [N, 8192] (n = 512c + 4p + t)."""
    a = out_core.reshape(NOCT, NCH, 128, 4, 2048)
    return a.transpose(1, 2, 3, 0, 4).reshape(N, JPC * N_OUT)


def kernel(x, C, W, grid):
    in_maps, scale_val = _host_prep(
        np.asarray(x, np.float32), np.asarray(C, np.float32),
        np.asarray(W, np.float32), np.asarray(grid, np.float32),
    )
    nc = _build_bass(scale_val)
    res = run_bass_kernel_spmd(nc, in_maps, core_ids=list(range(NCORES)))
    return np.ascontiguousarray(np.concatenate(
        [_assemble(r["out"]).astype(np.float32) for r in res.results], axis=1))


if __name__ == "__main__":
    rng = np.random.default_rng(0)
    x = rng.standard_normal((N, N_IN), dtype=np.float32)
    C = rng.standard_normal((11, N_IN * N_OUT), dtype=np.float32) * 0.005
    W = rng.standard_normal((1, N_IN * N_OUT), dtype=np.float32) * 0.005
    knots = -5.25 + 0.75 * np.arange(15, dtype=np.float32)
    grid = np.tile(knots, (N_IN, 1))
    out = kernel(x, C, W, grid)
    print("kernel out:", out.shape, out.dtype, float(np.abs(out).mean()))


# revision 27
# speedup vs baseline: 1.0170x; 1.0170x over previous
"""KAN layer (B-spline + silu) Trainium2 kernel, 8-way tensor-parallel.

Math (uniform knot grid):
  Truncated-power features S_i(v) = relu(v - i)^3, v = (x - t0)/h, i = 0..14,
  are computed in f32 on the scalar/vector engines (relu -> square -> cube).
  A small banded f32 matmul on the PE ("combine") turns them into the local
  B-spline basis  B_f = sum_{r=0..4} w5[r] * S_{f+r},  f = 0..10  (w5 =
  [1,-4,6,-4,1]/6).  The combine must run in f32: the truncated powers (up
  to ~2000) cancel down to B <= 0.67.  Its output is post-cancellation, so
  it is cast to fp16, and the main matmul runs fully in fp16 (1 PE
  cycle/row instead of 4 for f32):
      out[n, j*256+q] = sum_f B_f(v[n,j]) * Cw[f, j*256+q]
                        + silu(x[n,j]) * W[j*256+q],   Cw = C * W.
  fp16 scaling: weights are stored as 32*Cw / 32*W (lifting them out of the
  fp16 subnormal range) and the basis as B/32, silu/32 — the f32 PSUM result
  is the unscaled output.  The output is written to HBM in fp16 (halving the
  HBM-write floor, which dominates) and widened to f32 on the host.

Sharding: core s owns j in [32s, 32s+32) (columns [8192s, 8192(s+1)) of the
flattened output).  Per core, j's are grouped into 4 octets of 8; within an
octet, j-pairs map to the 4 PE row groups.  Row layout per 32-row group:
  S tile (f32):  [15 S(j_a), 15 S(j_b), silu'(j_a), silu'(j_b)]
  B tile (fp16): [11 B'(j_a), 11 B'(j_b), silu'(j_a), silu'(j_b), 8 zeros]
The combine matmul (K=32 -> M=32, tile_position (32r,32r), silu rows passed
through, last 8 out-cols zero) and the main matmul (K=32, rhs rows 24..31
zero, tile_position (32r,0)) both use full 32-row groups.

Performance structure (per core):
  - n is processed in 8 chunks of 256 rows; within a chunk, partition p
    holds output rows 256c + 2p + t (t = 0..1) — the main matmuls take
    stride-2 column slices of the B tile — so each store is 8 KB/partition
    (8 KB DMA packets, less per-packet overhead).
  - The four row-group matmuls of one parity issue back-to-back to distinct
    tile_positions (4-way concurrent on the PE); PSUM is one pool of four
    2-bank tiles shared by combine and main.
  - PSUM evacuation (f32 -> fp16 copy, the true output) alternates between
    scalar and vector; the chain is split (relu+square on scalar, cube on
    vector) to balance them, and chain ops run as half-tiles so they never
    displace more than ~1us of copies.
  - Input DMAs and octet 1-3 silu scatters ride the gpsimd queue, octet-0
    scatters the scalar queue, output stores own the sync queue.
  - Emission is a software-pipelined wavefront: chains 0 and 1 run entirely
    up-front (in engine time that is idle anyway while the first chunks
    compute), chain 2 is spread through octet 0, chain 3 through octet 1;
    combine piece q feeds main chunk q immediately.
"""

import numpy as np

import concourse.bass as bass
import concourse.bacc as bacc
import concourse.tile as tile
from concourse import mybir
from concourse.bass_utils import run_bass_kernel_spmd

N = 2048          # batch
N_IN = 256
N_OUT = 256
NCORES = 8
JPC = N_IN // NCORES      # 32 j per core
NOCT = JPC // 8           # 4 octets of 8 j's
NCH = N // 256            # 8 n-chunks of 256 rows
NQ = N // 512             # 4 combine pieces along n (piece q == chunk q)
F32 = mybir.dt.float32
F16 = mybir.dt.float16
WSCALE = 32.0             # fp16 weight scale (basis/silu carry 1/32)


def _build_bass(scale_val: float):
    nc = bacc.Bacc(trn_type="TRN2")

    xrep = nc.dram_tensor("xrep", [NOCT, 128, N], F16, kind="ExternalInput")
    biasv = nc.dram_tensor("biasv", [128, 1], F32, kind="ExternalInput")
    w5b = nc.dram_tensor("w5b", [128, 32], F32, kind="ExternalInput")
    rhsbd = nc.dram_tensor("rhsbd", [128, NOCT * 512], F16, kind="ExternalInput")
    siluT = nc.dram_tensor("siluT", [JPC, N], F32, kind="ExternalInput")
    # out[o, c, p, t*2048 + j] = row n = 256c + 2p + t, col 2048o + j
    out = nc.dram_tensor("out", [NOCT, NCH, 128, 4096], F16,
                         kind="ExternalOutput")

    with tile.TileContext(nc) as tc:
        with (
            tc.tile_pool(name="consts", bufs=1) as consts,
            tc.tile_pool(name="xin", bufs=4) as xin,
            tc.tile_pool(name="chain", bufs=2) as chain,
            tc.tile_pool(name="ss", bufs=1) as sspool,
            tc.tile_pool(name="bsb", bufs=1) as bpool,
            tc.tile_pool(name="stage", bufs=3) as stage_pool,
            tc.tile_pool(name="psum", bufs=4, space="PSUM") as psum_pool,
        ):
            # Input loads ride the gpsimd queue (sync = stores, scalar/vector
            # = compute + octet-0 scatters).
            bias_sb = consts.tile([128, 1], F32, name="bias_sb")
            nc.gpsimd.dma_start(out=bias_sb, in_=biasv[:, :])
            xr_tiles = []
            for o in range(NOCT):
                xr = xin.tile([128, N], F16, tag=f"xr{o}", name=f"xr{o}")
                xr_tiles.append(xr)
            nc.gpsimd.dma_start(out=xr_tiles[0], in_=xrep[0])
            nc.gpsimd.dma_start(out=xr_tiles[1], in_=xrep[1])
            rhs_sb = consts.tile([128, NOCT * 512], F16, name="rhs_sb")
            nc.gpsimd.dma_start(out=rhs_sb, in_=rhsbd[:, :])
            w5b_sb = consts.tile([128, 32], F32, name="w5b_sb")
            nc.gpsimd.dma_start(out=w5b_sb, in_=w5b[:, :])
            for o in range(2, NOCT):
                nc.gpsimd.dma_start(out=xr_tiles[o], in_=xrep[o])

            ss_tiles = [None] * NOCT
            b_tiles = [None] * NOCT
            chain_t = [None] * NOCT
            cnt = 0

            def emit_chain_step(o, step):
                # 6 half-tile steps ([128, 1024] each):
                #   0/1: relu halves (scalar), 2/3: square halves (scalar),
                #   4/5: cube halves (vector); silu scatters after step 5
                #   (gpsimd queue except octet 0).
                h = N // 2
                lo, hi = (0, h) if step % 2 == 0 else (h, N)
                if step < 2:
                    if step == 0:
                        t1 = chain.tile([128, N], F32, tag="t1", name=f"t1_{o}")
                        chain_t[o] = t1
                    t1 = chain_t[o]
                    nc.scalar.activation(
                        t1[:, lo:hi], xr_tiles[o][:, lo:hi],
                        mybir.ActivationFunctionType.Relu,
                        bias=bias_sb[:, 0:1], scale=scale_val,
                    )
                elif step < 4:
                    if step == 2:
                        t1 = chain_t[o]
                        t2 = chain.tile([128, N], F32, tag="t2", name=f"t2_{o}")
                        chain_t[o] = (t1, t2)
                    t1, t2 = chain_t[o]
                    nc.scalar.square(t2[:, lo:hi], t1[:, lo:hi])
                else:
                    if step == 4:
                        ss_tiles[o] = sspool.tile([128, N], F32, tag=f"ss{o}",
                                                  name=f"ss{o}")
                    t1, t2 = chain_t[o]
                    ss = ss_tiles[o]
                    nc.vector.tensor_mul(ss[:, lo:hi], t1[:, lo:hi],
                                         t2[:, lo:hi])
                    if step == 5:
                        eng = nc.scalar if o == 0 else nc.gpsimd
                        for r in range(4):
                            eng.dma_start(
                                out=ss[32 * r + 30 : 32 * r + 32, :],
                                in_=siluT[8 * o + 2 * r : 8 * o + 2 * r + 2, :],
                            )

            def emit_combine_piece(o, q):
                if q == 0:
                    b_tiles[o] = bpool.tile([128, N], F16, tag=f"b{o}",
                                            name=f"b{o}")
                bsb = b_tiles[o]
                bpt = psum_pool.tile([128, 1024], F32, tag="ps",
                                     name=f"bps{o}_{q}")
                bps = bpt[:, 0:512]
                for r in range(4):
                    nc.tensor.matmul(
                        bps[32 * r : 32 * r + 32, :],
                        lhsT=w5b_sb[32 * r : 32 * r + 32, :],
                        rhs=ss_tiles[o][32 * r : 32 * r + 32,
                                        512 * q : 512 * (q + 1)],
                        start=True,
                        stop=True,
                        tile_position=(32 * r, 32 * r),
                    )
                dst = bsb[:, 512 * q : 512 * (q + 1)]
                if q % 2 == 0:
                    nc.vector.tensor_scalar_mul(dst, bps, 1.0)
                else:
                    nc.scalar.copy(dst, bps)

            def emit_main_chunk(o, c):
                nonlocal cnt
                bsb = b_tiles[o]
                st = stage_pool.tile([128, 4096], F16, tag="st",
                                     name=f"st{o}_{c}")
                for t in range(2):       # row residue: n = 256c + 2p + t
                    for rp in range(2):  # row-group pairs (2rp, 2rp+1)
                        ps = psum_pool.tile([128, 1024], F32, tag="ps",
                                            name=f"ps{o}_{c}_{t}_{rp}")
                        for rr in range(2):
                            r = 2 * rp + rr
                            nc.tensor.matmul(
                                ps[:, 512 * rr : 512 * (rr + 1)],
                                lhsT=bsb[32 * r : 32 * r + 32,
                                         256 * c + t : 256 * (c + 1) : 2],
                                rhs=rhs_sb[32 * r : 32 * r + 32,
                                           512 * o : 512 * (o + 1)],
                                start=True,
                                stop=True,
                                tile_position=(32 * r, 0),
                            )
                        dst = st[:, 2048 * t + 1024 * rp :
                                 2048 * t + 1024 * (rp + 1)]
                        if cnt % 2 == 0:
                            nc.vector.tensor_scalar_mul(dst, ps, 1.0)
                        else:
                            nc.scalar.copy(dst, ps)
                        cnt += 1
                nc.sync.dma_start(out=out[o, c], in_=st)

            # Software-pipelined wavefront: chains 0/1 run up-front (in
            # otherwise-idle engine time), chain o+2 spreads over octet o's
            # second half; combine piece q feeds main chunks 2q/2q+1.
            for s in range(6):
                emit_chain_step(0, s)
            for s in range(6):
                emit_chain_step(1, s)
            for q in range(NQ):
                emit_combine_piece(0, q)
                emit_main_chunk(0, q)
            for o in range(NOCT):
                for c in range(4, NCH):
                    co = o + 2
                    if co < NOCT:
                        for s in (2 * (c - 4), 2 * (c - 4) + 1):
                            if s < 6:
                                emit_chain_step(co, s)
                    emit_main_chunk(o, c)
                if o + 1 < NOCT:
                    for q in range(NQ):
                        emit_combine_piece(o + 1, q)
                        emit_main_chunk(o + 1, q)

    nc.compile()
    return nc


def _host_prep(x, C, W, grid):
    """Build per-core input maps."""
    t0 = np.float64(grid[0, 0])
    h = np.float64(grid[0, 1] - grid[0, 0])
    w5 = np.array([1.0, -4.0, 6.0, -4.0, 1.0], np.float64) / 6.0

    # Banded combine weights (f32): B'_f = sum_r (w5[r]/32) S_{f+r} for both
    # j's of the pair, silu pass-through rows 30/31 -> 22/23, cols 24..31 = 0.
    w5b1 = np.zeros((32, 32), np.float32)
    for f in range(11):
        for r in range(5):
            w5b1[f + r, f] = np.float32(w5[r] / WSCALE)
            w5b1[15 + f + r, 11 + f] = np.float32(w5[r] / WSCALE)
    w5b1[30, 22] = 1.0
    w5b1[31, 23] = 1.0
    w5b = np.ascontiguousarray(np.tile(w5b1, (4, 1)))  # same block per group

    Cw32 = (C.astype(np.float64) * W.astype(np.float64) * WSCALE).astype(np.float16)
    W32 = (W.astype(np.float64) * WSCALE).astype(np.float16)

    xd = x.astype(np.float64)
    silu_p = (xd / (1.0 + np.exp(-xd)) / WSCALE).astype(np.float32)  # silu/32

    # S-tile partition layout within a 32-row group:
    #   s in [0,15)  -> S_i of j_a (i = s)
    #   s in [15,30) -> S_i of j_b (i = s - 15)
    #   s = 30/31    -> silu'(j_a)/silu'(j_b) (scatter; relu bias -64 ->
    #                   the chain writes exact zeros there first)
    s_idx = np.arange(128) % 32
    feat_i = np.where(s_idx < 15, s_idx, np.where(s_idx < 30, s_idx - 15, 0))
    which_b = np.where(s_idx < 15, 0, np.where(s_idx < 30, 1, s_idx - 30))
    biasv = np.where(
        s_idx < 30, -t0 / h - feat_i, -64.0
    ).astype(np.float32).reshape(128, 1)
    scale_val = float(np.float32(1.0 / h))

    x16 = x.astype(np.float16)
    in_maps = []
    for s in range(NCORES):
        jb = JPC * s
        xt = np.ascontiguousarray(x16[:, jb : jb + JPC].T)    # (32, N) fp16
        xrep = np.empty((NOCT, 128, N), np.float16)
        rgrp = np.arange(128) // 32
        for o in range(NOCT):
            jloc = 8 * o + 2 * rgrp + which_b
            xrep[o] = xt[jloc]
        silu_t = np.ascontiguousarray(silu_p[:, jb : jb + JPC].T)  # (32, N) f32

        # B-tile row layout per group: [11 B'a, 11 B'b, silu'a, silu'b, 8 pad]
        rhsbd = np.zeros((128, NOCT * 512), np.float16)
        for o in range(NOCT):
            for rr in range(4):
                ja = (jb + 8 * o + 2 * rr) * N_OUT
                jbc = (jb + 8 * o + 2 * rr + 1) * N_OUT
                base = 32 * rr
                rhsbd[base : base + 11, 512 * o : 512 * o + 256] = \
                    Cw32[:, ja : ja + 256]
                rhsbd[base + 11 : base + 22, 512 * o + 256 : 512 * o + 512] = \
                    Cw32[:, jbc : jbc + 256]
                rhsbd[base + 22, 512 * o : 512 * o + 256] = W32[0, ja : ja + 256]
                rhsbd[base + 23, 512 * o + 256 : 512 * o + 512] = \
                    W32[0, jbc : jbc + 256]
        in_maps.append({
            "xrep": np.ascontiguousarray(xrep),
            "biasv": biasv,
            "w5b": w5b,
            "rhsbd": np.ascontiguousarray(rhsbd),
            "siluT": silu_t,
        })
    return in_maps, scale_val


def _assemble(out_core):
    """[NOCT, NCH, 128, 4096] fp16 -> [N, 8192] (n = 256c + 2p + t)."""
    a = out_core.reshape(NOCT, NCH, 128, 2, 2048)
    return a.transpose(1, 2, 3, 0, 4).reshape(N, JPC * N_OUT)


def kernel(x, C, W, grid):
    in_maps, scale_val = _host_prep(
        np.asarray(x, np.float32), np.asarray(C, np.float32),
        np.asarray(W, np.float32), np.asarray(grid, np.float32),
    )
    nc = _build_bass(scale_val)
    res = run_bass_kernel_spmd(nc, in_maps, core_ids=list(range(NCORES)))
    return np.ascontiguousarray(np.concatenate(
        [_assemble(r["out"]).astype(np.float32) for r in res.results], axis=1))


if __name__ == "__main__":
    rng = np.random.default_rng(0)
    x = rng.standard_normal((N, N_IN), dtype=np.float32)
    C = rng.standard_normal((11, N_IN * N_OUT), dtype=np.float32) * 0.005
    W = rng.standard_normal((1, N_IN * N_OUT), dtype=np.float32) * 0.005
    knots = -5.25 + 0.75 * np.arange(15, dtype=np.float32)
    grid = np.tile(knots, (N_IN, 1))
    out = kernel(x, C, W, grid)
    print("kernel out:", out.shape, out.dtype, float(np.abs(out).mean()))
